# revision 34
# baseline (speedup 1.0000x reference)
"""Multi-head attention (B=16, S=1024, D=768, H=12) on 8 TRN2 NeuronCores.

Strategy: pure data parallelism — batch 16 is split 2-per-core; weights
replicated; no collectives. All matmul inputs bf16 (fp32 PSUM accum); the
host pre-transposes x/weights into d-major layouts whose leading dim is
the SBUF partition dim, so each load is one same-shape dma_start (the SP
sequencer pays ~0.6us per call), issued in need-order.

Per-core program (b in 0..1, head pairs hp in 0..5, unit u = 6b+hp):
  - qT2/kT2 [128, S]  two heads stacked on partitions (d-major, bf16),
    generated by matmul groups emitted as FILL work inside the previous
    unit's attention steps (no serial region at unit boundaries)
  - per step (qc, kt): two scores matmuls (one per head via tile_position
    row packing, streaming concurrently in disjoint row quadrants) into
    ONE [128,1024] PSUM tile, one ACT exp (scale=1/8) -> bf16 SBUF.
  - emission order per step: fills, PV(i-1), scores(i+1). PV_A carries
    the step's only semaphore wait (the exp data dep); every other PE
    instruction is wait-free, keeping its weight load overlapped with the
    previous matmul (a wait forces the load serial, ~+50ns/matmul).
  - PV: out[dh+1, q] += v_ext.T @ exp-half (row 64 accumulates the
    softmax denominator via a ones column in v).
  - fills: qkgen for unit u+1 (every unit), vgen (units 0-1 for batch 0,
    units 3-5 for batch 1), projection of batch b-1 (units 6-8); the
    final batch's projection runs j=0..4 inside unit 11 from step 7.
  - normalize (deferred into the next unit): copy denom row 64->0, then
    reciprocal_approx_fast, gpsimd partition_broadcast, DVE mult -> ao
    [d, t] bf16. Keep these on the DVE: generic tensor ops on gpsimd
    thrash its op library, and ACT copies thrash the activation table.
  - y = ao.T @ W_out^T + b_out; bias added by the DVE tensor_add during
    the PSUM->SBUF move (host-prebroadcast bias), not by K=1 matmuls.
"""
import ml_dtypes
import numpy as np
import concourse.bacc as bacc
import concourse.tile as tile
from concourse import mybir
from concourse.bass_utils import run_bass_kernel_spmd

FP32 = mybir.dt.float32
BF16 = mybir.dt.bfloat16
EXP = mybir.ActivationFunctionType.Exp

B, S, D, H = 2, 1024, 768, 12       # per-core batch of 2
HP = H // 2                          # head pairs (6)
DT = D // 128                        # d tiles (6)
KT = S // 128                        # k tiles (8)
QC = S // 512                        # q chunks (2)
TT = S // 128                        # t tiles per batch (8)
NU = B * HP                          # units (12)
N_CORES = 8

_CACHE = {}


def build_nc():
    nc = bacc.Bacc(trn_type="TRN2")
    # host-repacked layouts: leading dim 128 matches the SBUF partition dim so
    # each load is ONE same-shape dma_start (the SP sequencer pays ~0.6us per
    # dma_start; many small calls would serialize the startup)
    # x chunked by (batch, q-half) so each load is one contiguous-run call
    xT = nc.dram_tensor("xT", [128, B * QC, DT, 512], BF16, kind="ExternalInput")
    # wqkv columns permuted to [q_hp0|k_hp0|q_hp1|k_hp1|...|v] and split into
    # three tensors so each priority group is contiguous per partition
    wqa = nc.dram_tensor("wqa", [128, DT, 256], BF16, kind="ExternalInput")
    wqb = nc.dram_tensor("wqb", [128, DT, 2 * D - 256], BF16, kind="ExternalInput")
    wqc = nc.dram_tensor("wqc", [128, DT, D], BF16, kind="ExternalInput")
    woutT = nc.dram_tensor("woutT", [128, DT, D], BF16, kind="ExternalInput")
    bbc = nc.dram_tensor("bbc", [128, D], BF16, kind="ExternalInput")
    y = nc.dram_tensor("y", [B * S, D], FP32, kind="ExternalOutput")

    with tile.TileContext(nc) as tc:
        with (
            tc.tile_pool(name="wq", bufs=1) as p_wq,
            tc.tile_pool(name="wo", bufs=1) as p_wo,
            tc.tile_pool(name="cst", bufs=1) as p_cst,
            tc.tile_pool(name="xt", bufs=2) as p_xt,
            tc.tile_pool(name="vv", bufs=2) as p_v,
            tc.tile_pool(name="ao", bufs=2) as p_ao,
            tc.tile_pool(name="qk", bufs=4) as p_qk,
            tc.tile_pool(name="exp", bufs=4) as p_exp,
            tc.tile_pool(name="oc", bufs=8) as p_oc,
            tc.tile_pool(name="yy", bufs=10) as p_y,
            tc.tile_pool(name="rb", bufs=2) as p_rb,
            tc.tile_pool(name="r0", bufs=4) as p_r0,
            tc.tile_pool(name="sc", bufs=2, space="PSUM") as p_sc,
            tc.tile_pool(name="gen", bufs=2, space="PSUM") as p_gen,
            tc.tile_pool(name="oacc", bufs=2, space="PSUM") as p_oacc,
        ):
            wq = p_wq.tile([128, DT, 3 * D], BF16)
            wo = p_wo.tile([128, DT, D], BF16)
            bb = p_cst.tile([128, D], BF16)

            xts, vs, aos = {}, {}, {}

            # ---- DMA priority order ----
            xt0 = p_xt.tile([128, DT, S], BF16, tag="xt", name="xt0")
            xts[0] = xt0

            def load_xt(b):
                xt = p_xt.tile([128, DT, S], BF16, tag="xt")
                for qc in range(QC):
                    nc.sync.dma_start(
                        xt[:, :, qc * 512:(qc + 1) * 512],
                        xT[:, b * QC + qc],
                    )
                xts[b] = xt

            nc.sync.dma_start(wq[:, :, 0:256], wqa[:])       # unit-0 q/k cols
            nc.sync.dma_start(xt0[:, :, 0:512], xT[:, 0])     # x qc0
            nc.sync.dma_start(wq[:, :, 2 * D:3 * D], wqc[:])  # v cols
            nc.sync.dma_start(xt0[:, :, 512:1024], xT[:, 1])  # x qc1
            def load_rest():  # remaining q/k cols (qkgen(1) fills need them)
                nc.sync.dma_start(wq[:, :, 256:2 * D], wqb[:])

            def load_late():  # projection weights: first use is unit 6
                nc.sync.dma_start(wo[:, :, :], woutT[:])
                nc.sync.dma_start(bb[:], bbc[:])

            def alloc_v(b):
                v = p_v.tile([128, KT, H, 65], BF16, tag="vv")
                nc.vector.memset(v[:, :, :, 64], 1.0)
                vs[b] = v

            # ---- q/k generation (d-major bf16), per unit, as fill work ----
            qks = {}

            def ensure_qk(u):
                if u not in qks:
                    qks[u] = (
                        p_qk.tile([128, S], BF16, tag="qk", name=f"q{u}"),
                        p_qk.tile([128, S], BF16, tag="qk", name=f"k{u}"),
                    )
                return qks[u]

            def qkgen_group(u, part, qc):
                b2, hp2 = divmod(u, HP)

                def f():
                    tq = ensure_qk(u)[part]
                    qp = p_gen.tile([128, 512], FP32, tag="gen")
                    for j in range(DT):
                        nc.tensor.matmul(
                            qp[:],
                            wq[:, j,
                               256 * hp2 + 128 * part:256 * hp2 + 128 * (part + 1)],
                            xts[b2][:, j, qc * 512:(qc + 1) * 512],
                            start=(j == 0), stop=(j == DT - 1),
                        )
                    nc.vector.tensor_copy(tq[:, qc * 512:(qc + 1) * 512], qp[:])
                return f

            def qkgen_fills(u):
                return [
                    qkgen_group(u, 0, 0), qkgen_group(u, 1, 0),
                    qkgen_group(u, 1, 1), qkgen_group(u, 0, 1),
                ]

            # ---- v generation fills: one closure per (tt, head group) ----
            def vgen_fills(b, h0, nh):
                fills = []
                for tt in range(TT):
                    def f(tt=tt, h0=h0, nh=nh, b=b):
                        xt, v = xts[b], vs[b]
                        vp = p_gen.tile([128, 512], FP32, tag="gen")
                        cw = nh * 64
                        for j in range(DT):
                            nc.tensor.matmul(
                                vp[:, 0:cw],
                                xt[:, j, tt * 128:(tt + 1) * 128],
                                wq[:, j, 2 * D + h0 * 64:2 * D + h0 * 64 + cw],
                                start=(j == 0), stop=(j == DT - 1),
                            )
                        nc.vector.tensor_copy(
                            v[:, tt, h0:h0 + nh, 0:64],
                            vp[:, 0:cw].rearrange("p (h c) -> p h c", h=nh),
                        )
                    fills.append(f)
                return fills

            # ---- output projection fills (bias via DVE tensor_add) ----
            # jhi: contraction runs j=0..jhi-1; with jhi<DT the result is a
            # partial sum (+bias) kept in SBUF, finished later by proj_finish
            def proj_fills(b, jhi=DT, parts=None):
                fills = []
                for tt in range(TT):
                    box = {}
                    for ci, (c0, cw) in enumerate(((0, 512), (512, 256))):
                        def f(tt=tt, ci=ci, c0=c0, cw=cw, b=b, box=box):
                            ao = aos[b]
                            if ci == 0:
                                box["ys"] = p_y.tile([128, D], FP32, tag="yy", name=f"ys{b}_{tt}")
                                if parts is not None:
                                    parts[tt] = box["ys"]
                            ys = box["ys"]
                            yp = p_gen.tile([128, 512], FP32, tag="gen")
                            for j in range(jhi):
                                nc.tensor.matmul(
                                    yp[:, 0:cw],
                                    ao[:, j, tt * 128:(tt + 1) * 128],
                                    wo[:, j, c0:c0 + cw],
                                    start=(j == 0), stop=(j == jhi - 1),
                                )
                            nc.vector.tensor_add(
                                ys[:, c0:c0 + cw], yp[:, 0:cw], bb[:, c0:c0 + cw]
                            )
                            if jhi == DT and ci == 1:
                                nc.sync.dma_start(
                                    y[b * S + tt * 128:b * S + (tt + 1) * 128, :],
                                    ys[:],
                                )
                        fills.append(f)
                return fills

            def proj_finish(b, parts):
                # tail pass: uses the attention-free sc PSUM pool (4 bufs) and
                # splits the adds across DVE and GpSimd so neither chain gates
                # the PE
                fills = []
                for tt in range(TT):
                    for ci, (c0, cw) in enumerate(((0, 512), (512, 256))):
                        def f(tt=tt, ci=ci, c0=c0, cw=cw, b=b):
                            ao = aos[b]
                            ys = parts[tt]
                            yp = p_sc.tile([128, 512], FP32, tag="sc")
                            nc.tensor.matmul(
                                yp[:, 0:cw],
                                ao[:, DT - 1, tt * 128:(tt + 1) * 128],
                                wo[:, DT - 1, c0:c0 + cw],
                                start=True, stop=True,
                            )
                            nc.vector.tensor_add(
                                ys[:, c0:c0 + cw], yp[:, 0:cw], ys[:, c0:c0 + cw]
                            )
                            if ci == 1:
                                nc.sync.dma_start(
                                    y[b * S + tt * 128:b * S + (tt + 1) * 128, :],
                                    ys[:],
                                )
                        fills.append(f)
                return fills

            # ---- one attention unit (16 steps, fills interleaved) ----
            def unit(u, fills, pace=None, late=None, late2=None,
                     self_norm_qc0=False):
                b, hp = divmod(u, HP)
                v, ao = vs[b], aos[b]
                qT2, kT2 = ensure_qk(u)
                ocs, oaccs = {}, {}

                def scores_exp(qc, kt):
                    # both heads in one PSUM tile / one exp: the B-half PE
                    # instructions then carry no semaphore waits (their A
                    # twins already consumed them), which keeps their weight
                    # loads overlapped with the previous matmul
                    sc = p_sc.tile([128, 1024], FP32, tag="sc")
                    for half in range(2):
                        nc.tensor.matmul(
                            sc[:, 512 * half:512 * (half + 1)],
                            kT2[64 * half:64 * (half + 1),
                                kt * 128:(kt + 1) * 128],
                            qT2[64 * half:64 * (half + 1),
                                qc * 512:(qc + 1) * 512],
                            start=True, stop=True, tile_position=(64 * half, 0),
                        )
                    ex = p_exp.tile([128, 1024], BF16, tag="exp")
                    nc.scalar.activation(ex[:], sc[:], EXP, scale=0.125)
                    return ex

                def pv(qc, kt, half, ex):
                    key = (qc, half)
                    if kt == 0:
                        oaccs[key] = p_oacc.tile([65, 512], FP32, tag="oacc", name=f"oacc{key[0]}_{key[1]}")
                    nc.tensor.matmul(
                        oaccs[key][:], v[:, kt, 2 * hp + half, :],
                        ex[:, 512 * half:512 * (half + 1)],
                        start=(kt == 0), stop=(kt == KT - 1),
                    )
                    if kt == KT - 1:
                        # one copy frees the PSUM accumulator
                        oc = p_oc.tile([65, 512], FP32, tag="oc")
                        nc.vector.tensor_copy(oc[:], oaccs[key][:])
                        ocs[key] = oc

                def norm_one(qc, half):
                    def f():
                        # partition_broadcast only honors base-partition-0
                        # inputs on HW; the reciprocal's copy shifts the denom
                        # row 64 -> 0 (both are legal quadrant bases)
                        r0 = p_r0.tile([1, 512], FP32, tag="r0")
                        nc.vector.tensor_copy(r0[:], ocs[(qc, half)][64:65, :])
                        rr = p_r0.tile([1, 512], FP32, tag="rr")
                        nc.vector.reciprocal_approx_fast(rr[:], r0[:])
                        rb = p_rb.tile([64, 512], FP32, tag="rb")
                        nc.gpsimd.partition_broadcast(rb[:], rr[:])
                        nc.vector.tensor_mul(
                            ao[64 * half:64 * (half + 1), hp,
                               qc * 512:(qc + 1) * 512],
                            ocs[(qc, half)][0:64, :], rb[:],
                        )
                    return f

                fq = list(fills)
                steps = [(qc, kt) for qc in range(QC) for kt in range(KT)]
                exs = {0: scores_exp(*steps[0])}
                for i, (qc, kt) in enumerate(steps):
                    if i == 7 and late:
                        # deferred fill work whose dependencies (earlier
                        # cross-engine chains) only clear mid-unit
                        fq += late
                    if i == 9 and self_norm_qc0:
                        # qc0 accumulators were copied out during step 8
                        fq += [norm_one(0, 0), norm_one(0, 1)]
                    if i == 13 and late2:
                        fq += late2
                    n = pace if pace is not None else -(-len(fq) // (len(steps) - i))
                    for _ in range(min(n, len(fq))):
                        fq.pop(0)()
                    # PV first, then the NEXT step's scores: the scores
                    # matmul's PSUM-bank-reuse dependency is then already
                    # covered by the exp-sem wait PV_A just consumed, so it
                    # carries no wait and keeps its weight load overlapped
                    if i > 0:
                        pq, pk = steps[i - 1]
                        pv(pq, pk, 0, exs[i - 1])
                        pv(pq, pk, 1, exs[i - 1])
                    if i + 1 < len(steps):
                        exs[i + 1] = scores_exp(*steps[i + 1])
                pq, pk = steps[-1]
                pv(pq, pk, 0, exs[len(steps) - 1])
                pv(pq, pk, 1, exs[len(steps) - 1])
                for f in fq:
                    f()

                if self_norm_qc0:
                    return [norm_one(1, 0), norm_one(1, 1)]
                return [norm_one(0, 0), norm_one(0, 1),
                        norm_one(1, 0), norm_one(1, 1)]

            # ---- schedule ----
            alloc_v(0)
            load_rest()
            # prologue: q/k (qc0 halves) for unit 0; the qc1 halves and all
            # later units' q/k generation run as fill work
            ensure_qk(0)
            qkgen_group(0, 0, 0)()
            qkgen_group(0, 1, 0)()
            u0_late = [qkgen_group(0, 1, 1), qkgen_group(0, 0, 1)]

            vg1 = None
            norm = None
            for u in range(NU):
                b, hp = divmod(u, HP)
                if hp == 0:
                    aos[b] = p_ao.tile([128, DT, S], BF16, tag="ao", name=f"ao{b}")
                fills = []
                pace = None
                late = None
                late2 = None
                self_norm = False
                if u == 0:
                    # k-qc1 needed by step 4, q-qc1 by step 8, vgen(tt) by
                    # step tt+1; pace=2 front-loads to meet all deadlines
                    vg = vgen_fills(0, 0, 8)
                    fills += [u0_late[0], vg[0], u0_late[1]] + vg[1:]
                    fills += qkgen_fills(1)
                    pace = 2
                else:
                    if u + 1 < NU:
                        fills += qkgen_fills(u + 1)
                    fills += list(norm)
                    if u == 1:
                        fills += vgen_fills(0, 8, 4)
                    if u == 2:
                        load_xt(1)  # batch-1 x: ~3 units of DMA headroom
                        load_late()
                        alloc_v(1)
                        vg1 = vgen_fills(1, 0, 8) + vgen_fills(1, 8, 4)
                    if b == 0 and hp == 3:
                        fills += vg1[0:6]
                    if b == 0 and hp == 4:
                        fills += vg1[6:11]
                    if b == 0 and hp == 5:
                        fills += vg1[11:16]
                    if b == 1 and hp == 0:
                        pr0 = proj_fills(0)
                        fills += pr0[0:6]
                    if b == 1 and hp == 1:
                        fills += pr0[6:11]
                    if b == 1 and hp == 2:
                        fills += pr0[11:16]
                    if b == 1 and hp == 5:
                        # final-batch projection pass 1 (head pairs 0-4) fills
                        # the last unit from step 7, once the normalize(10)
                        # chain that finalizes the hp4 band has cleared; only
                        # the DT-1 band remains for the tail
                        parts1 = {}
                        late = proj_fills(1, jhi=DT - 1, parts=parts1)
                        fin = proj_finish(1, parts1)
                        late2 = fin[0:8]
                        self_norm = True
                        pace = 2
                norm = unit(u, fills, pace=pace, late=late, late2=late2,
                            self_norm_qc0=self_norm)
            for f in norm:
                f()
            for f in fin[8:16]:
                f()
    nc.finalize()
    return nc


def _repack(a):
    """[D, N] -> [128, DT, N]: row block j, row p -> (p, j)."""
    return np.ascontiguousarray(
        a.reshape(DT, 128, -1).transpose(1, 0, 2)
    ).astype(ml_dtypes.bfloat16)


def _marshal(x, W_qkv, W_out, b_out):
    W_qkv = np.asarray(W_qkv)
    # permute wqkv output dims to [q_hp0|k_hp0|q_hp1|k_hp1|...|v]
    perm = []
    for hp in range(HP):
        perm.extend(range(128 * hp, 128 * (hp + 1)))          # q cols of hp
        perm.extend(range(D + 128 * hp, D + 128 * (hp + 1)))  # k cols of hp
    perm.extend(range(2 * D, 3 * D))                          # v cols
    wqkvT = _repack(W_qkv[perm].T)
    wqa = np.ascontiguousarray(wqkvT[:, :, 0:256])
    wqb = np.ascontiguousarray(wqkvT[:, :, 256:2 * D])
    wqc = np.ascontiguousarray(wqkvT[:, :, 2 * D:3 * D])
    woutT = _repack(np.asarray(W_out).T)
    bbc = np.ascontiguousarray(
        np.broadcast_to(np.asarray(b_out).reshape(1, D), (128, D))
    ).astype(ml_dtypes.bfloat16)
    in_maps = []
    for c in range(N_CORES):
        xc = _repack(np.asarray(x)[B * c:B * (c + 1)].reshape(B * S, D).T)
        # [128, DT, B*S] -> [128, (b, qc), DT, 512]
        xq = np.ascontiguousarray(
            xc.reshape(128, DT, B * QC, 512).transpose(0, 2, 1, 3)
        )
        in_maps.append({"xT": xq, "wqa": wqa, "wqb": wqb, "wqc": wqc,
                        "woutT": woutT, "bbc": bbc})
    return in_maps


def run(x, W_qkv, W_out, b_out, trace=False, **spmd_kwargs):
    if "nc" not in _CACHE:
        _CACHE["nc"] = build_nc()
    nc = _CACHE["nc"]
    in_maps = _marshal(x, W_qkv, W_out, b_out)
    res = run_bass_kernel_spmd(
        nc, in_maps, core_ids=list(range(N_CORES)), trace=trace, **spmd_kwargs
    )
    out = np.stack([res.results[c]["y"] for c in range(N_CORES)], axis=0)
    out = out.reshape(N_CORES * B, S, D)
    return out, res


def kernel(x, W_qkv, W_out, b_out):
    out, _ = run(x, W_qkv, W_out, b_out)
    return out
